# revision 18
# baseline (speedup 1.0000x reference)
"""Trainium2 Bass kernel for nn_BoundaryTransformer_Block (2-layer causal
transformer + boundary-decision head), data-parallel over batch on 4 cores.

Strategy
--------
- Core c computes batch element c fully (B=4). No collectives (host-side
  measurements show collectives on this rig are host-bounced and ~ms-slow).
- All matmuls run as float32r (full PE rate at N>=256). Weights are
  pre-rounded to the f32r grid on host; on-device producers write f32r
  directly so the bir verifier accepts them as matmul operands.
- Residual stream kept in natural layout [128 tok, 512 hid] per 128-token
  tile; LayerNorm reduces along the free dim. The LN output is transposed
  via PE (identity matmul) into [hid, tok] for the QKV projections.
- Attention computed per (head, 512-query-half) with scores produced
  TRANSPOSED: S^T[kp, qp] = k^T(lhsT) x q^T(rhs). Softmax skips the max
  subtraction (scores are O(1) here); the row sum is obtained for free by
  augmenting V with a ones column (65th lhsT row of the av matmul).
- Causal structure: only lower-triangular 128x128 blocks are computed;
  the diagonal block is masked by an upper-triangular 0/1 multiply.
- LN gains/biases are folded into the following projection weights on the
  host; the V bias is folded into the output-projection bias (softmax rows
  sum to 1). Boundary mask = (out @ bd_w + bd_b > 0) exactly.
"""
import sys
import os
import numpy as np

for _p in ("/opt/trn_rl_repo",):
    if _p not in sys.path:
        sys.path.insert(0, _p)

import concourse.bass as bass
import concourse.mybir as mybir
import concourse.tile as tile
from concourse import bacc
from concourse.bass_utils import run_bass_kernel_spmd
from concourse.masks import make_identity, make_upper_triangular

B, L, HID, NH, HD, PROJ, NL = 4, 1024, 512, 8, 64, 2048, 2
LN_EPS = 1e-5
P = 128
TT = L // P        # 8 token tiles
C4 = HID // P      # 4 hidden chunks
PJT = PROJ // P    # 16 proj tiles
QKD = NH * HD      # 512
F32 = mybir.dt.float32
F32R = mybir.dt.float32r
N_CORES = 4
AF = mybir.ActivationFunctionType
ALU = mybir.AluOpType


def _round_f32r(a: np.ndarray) -> np.ndarray:
    """Round fp32 values to the fp32r grid (RNE at 12 mantissa LSBs)."""
    u = np.ascontiguousarray(a, dtype=np.float32).view(np.uint32)
    r = (u.astype(np.uint64) + 0x7FF + ((u >> 12) & 1)) & 0xFFFFF000
    return np.ascontiguousarray(r.astype(np.uint32)).view(np.float32)


def _layer_norm_to_hT(nc, sb, ps, xT_pool, out_sb, ident_r, eps_ap=None):
    """LN(out_sb) along free dim, write transposed result [128, C4, L] f32r.

    out_sb: [128, TT, HID] f32 residual. Returns hT tile [128, C4, TT*P] f32r.
    """
    hT = xT_pool.tile([P, C4, L], F32R, tag="xT")
    for t in range(TT):
        x = out_sb[:, t, :]
        stats = sb.tile([P, 6], F32, tag="lnstat")
        nc.vector.bn_stats(stats[:], x)
        mv = sb.tile([P, 2], F32, tag="lnmv")
        nc.vector.bn_aggr(mv[:], stats[:])
        sd = sb.tile([P, 1], F32, tag="lnsd")
        nc.scalar.activation(sd[:], mv[:, 1:2], AF.Sqrt, bias=eps_ap, scale=1.0)
        rstd = sb.tile([P, 1], F32, tag="lnrstd")
        nc.vector.reciprocal(rstd[:], sd[:])
        h = sb.tile([P, HID], F32R, tag="h")
        nc.vector.tensor_scalar(h[:], x, mv[:, 0:1], rstd[:],
                                op0=ALU.subtract, op1=ALU.mult)
        for c in range(C4):
            ptr = ps.tile([P, P], F32R, tag="ps_a")
            nc.tensor.transpose(ptr[:], h[:, c * P:(c + 1) * P], ident_r[:])
            nc.vector.tensor_copy(hT[:, c, t * P:(t + 1) * P], ptr[:])
    return hT


KSTAGE = int(os.environ.get("KSTAGE", "9"))


def _build(bd_b_val: float):
    nc = bacc.Bacc("TRN2", target_bir_lowering=False, debug=False,
                   num_devices=N_CORES)

    lx_d = nc.dram_tensor("lx", [L, HID], F32, kind="ExternalInput")
    mm_d = nc.dram_tensor("mmul", [P, TT], F32, kind="ExternalInput")
    bdw_d = nc.dram_tensor("bdw", [1, HID], F32, kind="ExternalInput")
    qkvw_d = nc.dram_tensor("qkvw", [NL, P, C4 * 3 * QKD], F32R, kind="ExternalInput")
    qkb_d = nc.dram_tensor("qkb", [P, NL * 8], F32, kind="ExternalInput")
    ow_d = nc.dram_tensor("ow", [NL, P, C4 * HID], F32R, kind="ExternalInput")
    fw1_d = nc.dram_tensor("fw1", [NL, PJT, P, C4 * P], F32R, kind="ExternalInput")
    fb1_d = nc.dram_tensor("fb1", [P, NL * PJT], F32, kind="ExternalInput")
    fw2_d = nc.dram_tensor("fw2", [NL, PROJ, HID], F32R, kind="ExternalInput")
    yout_d = nc.dram_tensor("yout", [L, HID], F32, kind="ExternalOutput")
    ymask_d = nc.dram_tensor("ymask", [P, TT], F32, kind="ExternalOutput")

    with tile.TileContext(nc) as tc:
        with tc.tile_pool(name="const", bufs=1) as cpool, \
             tc.tile_pool(name="sb", bufs=2) as sb, \
             tc.tile_pool(name="big", bufs=1) as big, \
             tc.tile_pool(name="xT", bufs=2) as xT_pool, \
             tc.tile_pool(name="wpool", bufs=1) as wpool, \
             tc.tile_pool(name="wstream", bufs=3) as wstream, \
             tc.tile_pool(name="pt", bufs=3) as ptile, \
             tc.tile_pool(name="ps", bufs=2, space="PSUM") as ps, \
             tc.tile_pool(name="psb", bufs=2, space="PSUM") as psb, \
             tc.tile_pool(name="ps4", bufs=1, space="PSUM") as ps4:

            # ---- constants ----
            ident = cpool.tile([P, P], F32)
            make_identity(nc, ident[:])
            mtri = cpool.tile([P, P], F32)
            make_upper_triangular(nc, mtri[:], val=1.0, diag=True)
            qkb_sb = cpool.tile([P, NL * 8], F32)
            nc.sync.dma_start(qkb_sb[:], qkb_d[:])
            fb1_sb = cpool.tile([P, NL * PJT], F32)
            nc.sync.dma_start(fb1_sb[:], fb1_d[:])
            bdw_sb = cpool.tile([1, HID], F32)
            nc.sync.dma_start(bdw_sb[:], bdw_d[:])
            bdw_bc = cpool.tile([P, HID], F32)
            nc.gpsimd.partition_broadcast(bdw_bc[:], bdw_sb[:])
            mm_sb = cpool.tile([P, TT], F32)
            nc.sync.dma_start(mm_sb[:], mm_d[:])
            ident_r = cpool.tile([P, P], F32R)
            nc.scalar.copy(ident_r[:], ident[:])
            eps_sb = cpool.tile([P, 1], F32)
            nc.vector.memset(eps_sb[:], LN_EPS)
            ones64_r = cpool.tile([P, HD], F32R)
            nc.scalar.activation(ones64_r[:], ident[:, 0:HD], AF.Identity,
                                 bias=1.0, scale=0.0)

            # ---- residual init ----
            out_sb = big.tile([P, TT, HID], F32, tag="resid")
            lx_r = lx_d.rearrange("(t p) n -> p t n", p=P)
            nc.sync.dma_start(out_sb[:], lx_r)

            for l in range(NL):
                # ---- layer weights ----
                qkvw_sb = wpool.tile([P, C4, 3 * QKD], F32R, tag="qkvw")
                nc.sync.dma_start(qkvw_sb[:].rearrange("p c n -> p (c n)"),
                                  qkvw_d[l])
                ow_sb = wpool.tile([P, C4, HID], F32R, tag="ow")
                nc.sync.dma_start(ow_sb[:].rearrange("p c n -> p (c n)"),
                                  ow_d[l])

                # ---- LN1 + transpose ----
                hT = _layer_norm_to_hT(nc, sb, ps, xT_pool, out_sb, ident_r, eps_sb[:])

                # ---- QKV projections ----
                # q^T, k^T: [128, 8, 1024] f32r; block m in 0..7 (q: 0-3, k: 4-7)
                qkT = big.tile([P, 8, L], F32R, tag="qkT")
                for m in (range(8) if KSTAGE >= 2 else []):
                    for half in range(2):
                        pq = ps.tile([P, 512], F32, tag="ps_a")
                        for c in range(C4):
                            nc.tensor.matmul(
                                pq[:], qkvw_sb[:, c, m * P:(m + 1) * P],
                                hT[:, c, half * 512:(half + 1) * 512],
                                start=(c == 0), stop=(c == C4 - 1))
                        nc.vector.tensor_copy(
                            qkT[:, m, half * 512:(half + 1) * 512], pq[:])
                # v (natural) + ones column: [128, NH, HD+1] per token tile
                v_aug = []
                for t in (range(TT) if KSTAGE >= 2 else []):
                    va = big.tile([P, NH, HD + 1], F32R, tag=f"vaug{t}")
                    pv = ps.tile([P, 512], F32, tag="ps_a")
                    for c in range(C4):
                        nc.tensor.matmul(
                            pv[:], hT[:, c, t * P:(t + 1) * P],
                            qkvw_sb[:, c, 2 * QKD:3 * QKD],
                            start=(c == 0), stop=(c == C4 - 1))
                    nc.vector.tensor_copy(
                        va[:, :, 0:HD], pv[:].rearrange("p (h d) -> p h d", h=NH))
                    nc.scalar.activation(va[:, :, HD:HD + 1], mtri[:, 0:NH],
                                         AF.Identity, bias=1.0, scale=0.0)
                    v_aug.append(va)

                # ---- attention ----
                avT = xT_pool.tile([P, C4, L], F32R, tag="xT")

                def emit_norm(h, qh, pav):
                    # normalize av rows by the ones-row sum; deferred by one
                    # (h, qh) iteration so its latency hides under the next
                    # head's matmuls.
                    r0 = (h % 2) * HD
                    mb = h // 2
                    srow = sb.tile([P, 512], F32, tag="srow")
                    nc.vector.tensor_copy(srow[HD:HD + 1, :], pav[HD:HD + 1, :])
                    sq = sb.tile([P, 4], F32, tag="sq")
                    nc.sync.dma_start(sq[:], srow[HD:HD + 1, :])
                    sqr = sb.tile([P, 4], F32, tag="sqr")
                    nc.vector.reciprocal(sqr[:], sq[:])
                    srow0 = sb.tile([1, 512], F32, tag="srow0")
                    nc.sync.dma_start(srow0[:], sqr[:])
                    bc_sb = sb.tile([HD, 512], F32, tag="bcsb")
                    nc.gpsimd.partition_broadcast(bc_sb[:], srow0[:])
                    if r0 == 0:
                        nc.vector.tensor_mul(
                            avT[0:HD, mb, qh * 512:(qh + 1) * 512],
                            pav[0:HD, :], bc_sb[:])
                    else:
                        # DVE is lane-locked; compute at base 0 and let a
                        # SBUF->SBUF DMA shift the result up 64 partitions.
                        avtmp = sb.tile([HD, 512], F32R, tag="avtmp")
                        nc.vector.tensor_mul(avtmp[:], pav[0:HD, :], bc_sb[:])
                        nc.sync.dma_start(
                            avT[HD:P, mb, qh * 512:(qh + 1) * 512], avtmp[:])

                pending = None
                for h in (range(NH) if KSTAGE >= 3 else []):
                    r0 = (h % 2) * HD        # partition offset of this head
                    mb = h // 2              # 128-block of q/k dims
                    for qh in range(2):
                        pav = psb.tile([P, 512], F32, tag="pav")
                        jmax = 4 * (qh + 1)

                        def emit_S(j):
                            qlo = max(j - 4 * qh, 0) * P      # local col start
                            n = 512 - qlo
                            pS = ps.tile([P, 512], F32, tag="ps_a")
                            nc.tensor.matmul(
                                pS[:, 0:n],
                                qkT[r0:r0 + HD, 4 + mb, j * P:(j + 1) * P],
                                qkT[r0:r0 + HD, mb, qh * 512 + qlo:(qh + 1) * 512],
                                start=True, stop=True)
                            return pS, qlo, n

                        def emit_av(j, pS, qlo, n):
                            pt_ = ptile.tile([P, 512], F32R, tag="ptile")
                            nc.scalar.activation(pt_[:, 0:n], pS[:, 0:n],
                                                 AF.Exp, scale=float(1.0 / np.sqrt(HD)))
                            if qh == 0 or j >= 4:
                                nc.vector.tensor_mul(pt_[:, 0:P], pt_[:, 0:P],
                                                     mtri[:])
                            nc.tensor.matmul(
                                pav[0:HD + 1, qlo:512],
                                v_aug[j][:, h, :], pt_[:, 0:n],
                                start=(j == 0), stop=(j == jmax - 1))

                        prev = None
                        for j in range(jmax):
                            cur = (j,) + emit_S(j)
                            if prev is not None:
                                emit_av(*prev)
                            prev = cur
                        emit_av(*prev)
                        if pending is not None:
                            emit_norm(*pending)
                        pending = (h, qh, pav)
                if pending is not None:
                    emit_norm(*pending)
                    pending = None

                # ---- output projection + residual ----
                for t in (range(TT) if KSTAGE >= 4 else []):
                    po = ps.tile([P, 512], F32, tag="ps_a")
                    for c in range(C4):
                        nc.tensor.matmul(po[:], avT[:, c, t * P:(t + 1) * P],
                                         ow_sb[:, c, :],
                                         start=(c == 0), stop=(c == C4 - 1))
                    nc.vector.tensor_add(out_sb[:, t, :], out_sb[:, t, :], po[:])

                # ---- LN2 + transpose ----
                h2T = _layer_norm_to_hT(nc, sb, ps, xT_pool, out_sb, ident_r, eps_sb[:])

                # ---- FFN ----
                for g in (range(2) if KSTAGE >= 5 else []):
                    pf2 = ps4.tile([P, 4, 512], F32, tag="ps_out2")
                    for p_ in range(PJT):
                        w1 = wstream.tile([P, C4, P], F32R, tag="w1")
                        nc.sync.dma_start(
                            w1[:].rearrange("p c n -> p (c n)"), fw1_d[l, p_])
                        w2 = wstream.tile([P, HID], F32R, tag="w2")
                        nc.sync.dma_start(w2[:], fw2_d[l, p_ * P:(p_ + 1) * P, :])
                        pf = ps.tile([P, 512], F32, tag="ps_a")
                        for c in range(C4):
                            nc.tensor.matmul(
                                pf[:], w1[:, c, :],
                                h2T[:, c, g * 512:(g + 1) * 512],
                                start=(c == 0), stop=(c == C4 - 1))
                        relu = ptile.tile([P, 512], F32R, tag="relu")
                        nc.scalar.activation(
                            relu[:], pf[:], AF.Relu,
                            bias=fb1_sb[:, l * PJT + p_:l * PJT + p_ + 1])
                        for th in range(4):
                            nc.tensor.matmul(
                                pf2[:, th, :], relu[:, th * P:(th + 1) * P],
                                w2[:], start=(p_ == 0), stop=(p_ == PJT - 1))
                    for th in range(4):
                        t = g * 4 + th
                        nc.vector.tensor_add(out_sb[:, t, :], out_sb[:, t, :],
                                             pf2[:, th, :])

            # ---- boundary head + blend ----
            yout_r = yout_d.rearrange("(t p) n -> p t n", p=P)
            for t in (range(TT) if KSTAGE >= 6 else []):
                lxt = sb.tile([P, HID], F32, tag="lxt")
                nc.sync.dma_start(lxt[:], lx_r[:, t, :])
                if KSTAGE >= 7:
                    z = sb.tile([P, 1], F32, tag="z")
                    zjunk = sb.tile([P, HID], F32, tag="diff")
                    nc.vector.scalar_tensor_tensor(
                        zjunk[:], out_sb[:, t, :], 1.0, bdw_bc[:],
                        op0=ALU.mult, op1=ALU.mult, accum_out=z[:])
                    msk = sb.tile([P, 1], F32, tag="msk")
                    if KSTAGE >= 8:
                        nc.vector.tensor_single_scalar(
                            msk[:], z[:], float(-bd_b_val), op=ALU.is_gt)
                    else:
                        nc.vector.tensor_copy(msk[:], z[:])
                    nc.sync.dma_start(ymask_d[:, t:t + 1], msk[:])
                diff = sb.tile([P, HID], F32, tag="diff")
                nc.vector.tensor_sub(diff[:], out_sb[:, t, :], lxt[:])
                res = sb.tile([P, HID], F32, tag="res")
                nc.vector.scalar_tensor_tensor(
                    res[:], diff[:], mm_sb[:, t:t + 1], lxt[:],
                    op0=ALU.mult, op1=ALU.add)
                nc.sync.dma_start(yout_r[:, t, :], res[:])

    nc.compile()
    return nc


_CACHE: dict = {}


def kernel(layer_x, full_mask, mask_multiplier, qkv_w, qkv_b, o_w, o_b,
           ln1_g, ln1_b, ff_w1, ff_b1, ff_w2, ff_b2, ln2_g, ln2_b,
           bd_w, bd_b):
    layer_x = np.asarray(layer_x, dtype=np.float32)
    mask_multiplier = np.asarray(mask_multiplier, dtype=np.float32)
    qkv_w = np.asarray(qkv_w, dtype=np.float32)
    qkv_b = np.asarray(qkv_b, dtype=np.float32)
    o_w = np.asarray(o_w, dtype=np.float32)
    o_b = np.asarray(o_b, dtype=np.float32)
    ln1_g = np.asarray(ln1_g, dtype=np.float32)
    ln1_b = np.asarray(ln1_b, dtype=np.float32)
    ff_w1 = np.asarray(ff_w1, dtype=np.float32)
    ff_b1 = np.asarray(ff_b1, dtype=np.float32)
    ff_w2 = np.asarray(ff_w2, dtype=np.float32)
    ff_b2 = np.asarray(ff_b2, dtype=np.float32)
    ln2_g = np.asarray(ln2_g, dtype=np.float32)
    ln2_b = np.asarray(ln2_b, dtype=np.float32)
    bd_w = np.asarray(bd_w, dtype=np.float32)
    bd_b = np.asarray(bd_b, dtype=np.float32)

    # host-side folding (exact in fp32/fp64 where it matters)
    qkv_w_eff = np.empty_like(qkv_w)
    qkv_b_eff = np.empty_like(qkv_b)
    fw1_eff = np.empty_like(ff_w1)
    fb1_eff = np.empty_like(ff_b1)
    o_b_eff = np.empty_like(o_b)
    for i in range(NL):
        qkv_w_eff[i] = ln1_g[i][:, None] * qkv_w[i]
        qkv_b_eff[i] = qkv_b[i] + ln1_b[i] @ qkv_w[i]
        fw1_eff[i] = ln2_g[i][:, None] * ff_w1[i]
        fb1_eff[i] = ff_b1[i] + ln2_b[i] @ ff_w1[i]
        # V bias folds through the softmax (rows sum to 1) into o_b
        o_b_eff[i] = o_b[i] + qkv_b_eff[i][2 * QKD:] @ o_w[i]
    if np.any(o_b_eff != 0.0) or np.any(ff_b2 != 0.0):
        raise NotImplementedError(
            "nonzero output-projection/FFN2 bias not supported by this kernel")

    qkb_host = np.concatenate(
        [qkv_b_eff[i][:2 * QKD].reshape(8, P).T for i in range(NL)],
        axis=1).astype(np.float32)                      # [128, NL*8]
    fb1_host = np.concatenate(
        [fb1_eff[i].reshape(PJT, P).T for i in range(NL)],
        axis=1).astype(np.float32)                      # [128, NL*16]

    key = float(bd_b[0])
    if key not in _CACHE:
        _CACHE[key] = _build(key)
    nc = _CACHE[key]

    def pmajor(w):
        # [HID, N] -> [128, C4*N] partition-major for contiguous DMA
        n = w.shape[1]
        return np.ascontiguousarray(
            w.reshape(C4, P, n).transpose(1, 0, 2).reshape(P, C4 * n))

    qkvw_h = np.stack([pmajor(qkv_w_eff[i]) for i in range(NL)])
    ow_h = np.stack([pmajor(o_w[i]) for i in range(NL)])
    fw1_h = np.stack([
        np.stack([pmajor(fw1_eff[i][:, p * P:(p + 1) * P]) for p in range(PJT)])
        for i in range(NL)])
    shared = {
        "bdw": np.ascontiguousarray(bd_w.reshape(1, HID)),
        "qkvw": _round_f32r(qkvw_h),
        "qkb": np.ascontiguousarray(qkb_host),
        "ow": _round_f32r(ow_h),
        "fw1": _round_f32r(fw1_h),
        "fb1": np.ascontiguousarray(fb1_host),
        "fw2": _round_f32r(ff_w2),
    }
    in_maps = []
    for b in range(B):
        m = dict(shared)
        m["lx"] = np.ascontiguousarray(layer_x[b])
        m["mmul"] = np.ascontiguousarray(
            mask_multiplier[b, :, 0].reshape(TT, P).T)
        in_maps.append(m)

    res = run_bass_kernel_spmd(nc, in_maps, core_ids=list(range(N_CORES)))
    global LAST_RESULT
    LAST_RESULT = res
    new_out = np.stack([res.results[b]["yout"] for b in range(B)])
    new_mask = np.stack(
        [res.results[b]["ymask"].T.reshape(L, 1) for b in range(B)])
    return new_out, new_mask


# revision 20
# speedup vs baseline: 1.0501x; 1.0501x over previous
"""Trainium2 Bass kernel for nn_BoundaryTransformer_Block (2-layer causal
transformer + boundary-decision head), data-parallel over batch on 4 cores.

Strategy
--------
- Core c computes batch element c fully (B=4). No collectives (host-side
  measurements show collectives on this rig are host-bounced and ~ms-slow).
- All matmuls run as float32r (full PE rate at N>=256). Weights are
  pre-rounded to the f32r grid on host; on-device producers write f32r
  directly so the bir verifier accepts them as matmul operands.
- Residual stream kept in natural layout [128 tok, 512 hid] per 128-token
  tile; LayerNorm reduces along the free dim. The LN output is transposed
  via PE (identity matmul) into [hid, tok] for the QKV projections.
- Attention computed per (head, 512-query-half) with scores produced
  TRANSPOSED: S^T[kp, qp] = k^T(lhsT) x q^T(rhs). Softmax skips the max
  subtraction (scores are O(1) here); the row sum is obtained for free by
  augmenting V with a ones column (65th lhsT row of the av matmul).
- Causal structure: only lower-triangular 128x128 blocks are computed;
  the diagonal block is masked by an upper-triangular 0/1 multiply.
- LN gains/biases are folded into the following projection weights on the
  host; the V bias is folded into the output-projection bias (softmax rows
  sum to 1). Boundary mask = (out @ bd_w + bd_b > 0) exactly.
"""
import sys
import os
import numpy as np

for _p in ("/opt/trn_rl_repo",):
    if _p not in sys.path:
        sys.path.insert(0, _p)

import concourse.bass as bass
import concourse.mybir as mybir
import concourse.tile as tile
from concourse import bacc
from concourse.bass_utils import run_bass_kernel_spmd
from concourse.masks import make_identity, make_upper_triangular

B, L, HID, NH, HD, PROJ, NL = 4, 1024, 512, 8, 64, 2048, 2
LN_EPS = 1e-5
P = 128
TT = L // P        # 8 token tiles
C4 = HID // P      # 4 hidden chunks
PJT = PROJ // P    # 16 proj tiles
QKD = NH * HD      # 512
F32 = mybir.dt.float32
F32R = mybir.dt.float32r
N_CORES = 4
AF = mybir.ActivationFunctionType
ALU = mybir.AluOpType


def _round_f32r(a: np.ndarray) -> np.ndarray:
    """Round fp32 values to the fp32r grid (RNE at 12 mantissa LSBs)."""
    u = np.ascontiguousarray(a, dtype=np.float32).view(np.uint32)
    r = (u.astype(np.uint64) + 0x7FF + ((u >> 12) & 1)) & 0xFFFFF000
    return np.ascontiguousarray(r.astype(np.uint32)).view(np.float32)


def _layer_norm_to_hT(nc, sb, ps, xT_pool, out_sb, ident_r, eps_ap=None):
    """LN(out_sb) along free dim, write transposed result [128, C4, L] f32r.

    out_sb: [128, TT, HID] f32 residual. Returns hT tile [128, C4, TT*P] f32r.
    """
    hT = xT_pool.tile([P, C4, L], F32R, tag="xT")
    for t in range(TT):
        x = out_sb[:, t, :]
        stats = sb.tile([P, 6], F32, tag="lnstat")
        nc.vector.bn_stats(stats[:], x)
        mv = sb.tile([P, 2], F32, tag="lnmv")
        nc.vector.bn_aggr(mv[:], stats[:])
        sd = sb.tile([P, 1], F32, tag="lnsd")
        nc.scalar.activation(sd[:], mv[:, 1:2], AF.Sqrt, bias=eps_ap, scale=1.0)
        rstd = sb.tile([P, 1], F32, tag="lnrstd")
        nc.vector.reciprocal(rstd[:], sd[:])
        h = sb.tile([P, HID], F32R, tag="h")
        nc.vector.tensor_scalar(h[:], x, mv[:, 0:1], rstd[:],
                                op0=ALU.subtract, op1=ALU.mult)
        for c in range(C4):
            ptr = ps.tile([P, P], F32R, tag="ps_a")
            nc.tensor.transpose(ptr[:], h[:, c * P:(c + 1) * P], ident_r[:])
            nc.vector.tensor_copy(hT[:, c, t * P:(t + 1) * P], ptr[:])
    return hT


KSTAGE = int(os.environ.get("KSTAGE", "9"))


def _build(bd_b_val: float):
    nc = bacc.Bacc("TRN2", target_bir_lowering=False, debug=False,
                   num_devices=N_CORES)

    lx_d = nc.dram_tensor("lx", [L, HID], F32, kind="ExternalInput")
    mm_d = nc.dram_tensor("mmul", [P, TT], F32, kind="ExternalInput")
    bdw_d = nc.dram_tensor("bdw", [1, HID], F32, kind="ExternalInput")
    qkvw_d = nc.dram_tensor("qkvw", [NL, P, C4 * 3 * QKD], F32R, kind="ExternalInput")
    qkb_d = nc.dram_tensor("qkb", [P, NL * 8], F32, kind="ExternalInput")
    ow_d = nc.dram_tensor("ow", [NL, P, C4 * HID], F32R, kind="ExternalInput")
    fw1_d = nc.dram_tensor("fw1", [NL, PJT, P, C4 * P], F32R, kind="ExternalInput")
    fb1_d = nc.dram_tensor("fb1", [P, NL * PJT], F32, kind="ExternalInput")
    fw2_d = nc.dram_tensor("fw2", [NL, PROJ, HID], F32R, kind="ExternalInput")
    yout_d = nc.dram_tensor("yout", [L, HID], F32, kind="ExternalOutput")
    ymask_d = nc.dram_tensor("ymask", [P, TT], F32, kind="ExternalOutput")

    with tile.TileContext(nc) as tc:
        with tc.tile_pool(name="const", bufs=1) as cpool, \
             tc.tile_pool(name="sb", bufs=2) as sb, \
             tc.tile_pool(name="big", bufs=1) as big, \
             tc.tile_pool(name="xT", bufs=2) as xT_pool, \
             tc.tile_pool(name="wpool", bufs=1) as wpool, \
             tc.tile_pool(name="wstream", bufs=3) as wstream, \
             tc.tile_pool(name="pt", bufs=3) as ptile, \
             tc.tile_pool(name="ps", bufs=2, space="PSUM") as ps, \
             tc.tile_pool(name="psb", bufs=2, space="PSUM") as psb, \
             tc.tile_pool(name="ps4", bufs=1, space="PSUM") as ps4:

            # ---- constants ----
            ident = cpool.tile([P, P], F32)
            make_identity(nc, ident[:])
            mtri = cpool.tile([P, P], F32)
            make_upper_triangular(nc, mtri[:], val=1.0, diag=True)
            qkb_sb = cpool.tile([P, NL * 8], F32)
            nc.sync.dma_start(qkb_sb[:], qkb_d[:])
            fb1_sb = cpool.tile([P, NL * PJT], F32)
            nc.sync.dma_start(fb1_sb[:], fb1_d[:])
            bdw_sb = cpool.tile([1, HID], F32)
            nc.sync.dma_start(bdw_sb[:], bdw_d[:])
            bdw_bc = cpool.tile([P, HID], F32)
            nc.gpsimd.partition_broadcast(bdw_bc[:], bdw_sb[:])
            mm_sb = cpool.tile([P, TT], F32)
            nc.sync.dma_start(mm_sb[:], mm_d[:])
            ident_r = cpool.tile([P, P], F32R)
            nc.scalar.copy(ident_r[:], ident[:])
            eps_sb = cpool.tile([P, 1], F32)
            nc.vector.memset(eps_sb[:], LN_EPS)
            ones64_r = cpool.tile([P, HD], F32R)
            nc.scalar.activation(ones64_r[:], ident[:, 0:HD], AF.Identity,
                                 bias=1.0, scale=0.0)

            # ---- residual init ----
            out_sb = big.tile([P, TT, HID], F32, tag="resid")
            lx_r = lx_d.rearrange("(t p) n -> p t n", p=P)
            nc.sync.dma_start(out_sb[:], lx_r)

            for l in range(NL):
                # ---- layer weights ----
                qkvw_sb = wpool.tile([P, C4, 3 * QKD], F32R, tag="qkvw")
                nc.sync.dma_start(qkvw_sb[:].rearrange("p c n -> p (c n)"),
                                  qkvw_d[l])
                ow_sb = wpool.tile([P, C4, HID], F32R, tag="ow")
                nc.sync.dma_start(ow_sb[:].rearrange("p c n -> p (c n)"),
                                  ow_d[l])

                # ---- LN1 + transpose ----
                hT = _layer_norm_to_hT(nc, sb, ps, xT_pool, out_sb, ident_r, eps_sb[:])

                # ---- QKV projections ----
                # q^T, k^T: [128, 8, 1024] f32r; block m in 0..7 (q: 0-3, k: 4-7)
                qkT = big.tile([P, 8, L], F32R, tag="qkT")
                for m in (range(8) if KSTAGE >= 2 else []):
                    for half in range(2):
                        pq = ps.tile([P, 512], F32, tag="ps_a")
                        for c in range(C4):
                            nc.tensor.matmul(
                                pq[:], qkvw_sb[:, c, m * P:(m + 1) * P],
                                hT[:, c, half * 512:(half + 1) * 512],
                                start=(c == 0), stop=(c == C4 - 1))
                        nc.vector.tensor_copy(
                            qkT[:, m, half * 512:(half + 1) * 512], pq[:])
                # v (natural) + ones column: [128, NH, HD+1] per token tile
                v_aug = []
                for t in (range(TT) if KSTAGE >= 2 else []):
                    va = big.tile([P, NH, HD + 1], F32R, tag=f"vaug{t}")
                    pv = ps.tile([P, 512], F32, tag="ps_a")
                    for c in range(C4):
                        nc.tensor.matmul(
                            pv[:], hT[:, c, t * P:(t + 1) * P],
                            qkvw_sb[:, c, 2 * QKD:3 * QKD],
                            start=(c == 0), stop=(c == C4 - 1))
                    nc.vector.tensor_copy(
                        va[:, :, 0:HD], pv[:].rearrange("p (h d) -> p h d", h=NH))
                    nc.scalar.activation(va[:, :, HD:HD + 1], mtri[:, 0:NH],
                                         AF.Identity, bias=1.0, scale=0.0)
                    v_aug.append(va)

                # ---- attention ----
                avT = xT_pool.tile([P, C4, L], F32R, tag="xT")

                def emit_norm(h, qh, pav):
                    # normalize av rows by the ones-row sum; deferred by one
                    # (h, qh) iteration so its latency hides under the next
                    # head's matmuls.
                    r0 = (h % 2) * HD
                    mb = h // 2
                    srow = sb.tile([P, 512], F32, tag="srow")
                    nc.vector.tensor_copy(srow[HD:HD + 1, :], pav[HD:HD + 1, :])
                    sq = sb.tile([P, 4], F32, tag="sq")
                    nc.sync.dma_start(sq[:], srow[HD:HD + 1, :])
                    sqr = sb.tile([P, 4], F32, tag="sqr")
                    nc.vector.reciprocal(sqr[:], sq[:])
                    srow0 = sb.tile([1, 512], F32, tag="srow0")
                    nc.sync.dma_start(srow0[:], sqr[:])
                    bc_sb = sb.tile([HD, 512], F32, tag="bcsb")
                    nc.gpsimd.partition_broadcast(bc_sb[:], srow0[:])
                    if r0 == 0:
                        nc.vector.tensor_mul(
                            avT[0:HD, mb, qh * 512:(qh + 1) * 512],
                            pav[0:HD, :], bc_sb[:])
                    else:
                        # DVE is lane-locked; compute at base 0 and let a
                        # SBUF->SBUF DMA shift the result up 64 partitions.
                        avtmp = sb.tile([HD, 512], F32R, tag="avtmp")
                        nc.vector.tensor_mul(avtmp[:], pav[0:HD, :], bc_sb[:])
                        nc.sync.dma_start(
                            avT[HD:P, mb, qh * 512:(qh + 1) * 512], avtmp[:])

                pending = None
                for h in (range(NH) if KSTAGE >= 3 else []):
                    r0 = (h % 2) * HD        # partition offset of this head
                    mb = h // 2              # 128-block of q/k dims
                    for qh in range(2):
                        pav = psb.tile([P, 512], F32, tag="pav")
                        jmax = 4 * (qh + 1)

                        def emit_S(j):
                            qlo = max(j - 4 * qh, 0) * P      # local col start
                            n = 512 - qlo
                            pS = ps.tile([P, 512], F32, tag="ps_a")
                            nc.tensor.matmul(
                                pS[:, 0:n],
                                qkT[r0:r0 + HD, 4 + mb, j * P:(j + 1) * P],
                                qkT[r0:r0 + HD, mb, qh * 512 + qlo:(qh + 1) * 512],
                                start=True, stop=True)
                            return pS, qlo, n

                        def emit_av(j, pS, qlo, n):
                            pt_ = ptile.tile([P, 512], F32R, tag="ptile")
                            nc.scalar.activation(pt_[:, 0:n], pS[:, 0:n],
                                                 AF.Exp, scale=float(1.0 / np.sqrt(HD)))
                            if qh == 0 or j >= 4:
                                nc.vector.tensor_mul(pt_[:, 0:P], pt_[:, 0:P],
                                                     mtri[:])
                            nc.tensor.matmul(
                                pav[0:HD + 1, qlo:512],
                                v_aug[j][:, h, :], pt_[:, 0:n],
                                start=(j == 0), stop=(j == jmax - 1))

                        prev = None
                        for j in range(jmax):
                            cur = (j,) + emit_S(j)
                            if prev is not None:
                                emit_av(*prev)
                            prev = cur
                        emit_av(*prev)
                        if pending is not None:
                            emit_norm(*pending)
                        pending = (h, qh, pav)
                if pending is not None:
                    emit_norm(*pending)
                    pending = None

                # ---- output projection + residual ----
                for t in (range(TT) if KSTAGE >= 4 else []):
                    po = ps.tile([P, 512], F32, tag="ps_a")
                    for c in range(C4):
                        nc.tensor.matmul(po[:], avT[:, c, t * P:(t + 1) * P],
                                         ow_sb[:, c, :],
                                         start=(c == 0), stop=(c == C4 - 1))
                    nc.vector.tensor_add(out_sb[:, t, :], out_sb[:, t, :], po[:])

                # ---- LN2 + transpose ----
                h2T = _layer_norm_to_hT(nc, sb, ps, xT_pool, out_sb, ident_r, eps_sb[:])

                # ---- FFN ----
                groups = [(0, 4), (4, 4)]
                for g0, gn in (groups if KSTAGE >= 5 else []):
                    pf2 = ps4.tile([P, 4, 512], F32, tag="ps_out2")
                    for p_ in range(PJT):
                        w1 = wstream.tile([P, C4, P], F32R, tag="w1")
                        nc.sync.dma_start(
                            w1[:].rearrange("p c n -> p (c n)"), fw1_d[l, p_])
                        w2 = wstream.tile([P, HID], F32R, tag="w2")
                        nc.sync.dma_start(w2[:], fw2_d[l, p_ * P:(p_ + 1) * P, :])
                        pf = ps.tile([P, 512], F32, tag="ps_a")
                        nw = gn * P
                        for c in range(C4):
                            nc.tensor.matmul(
                                pf[:, 0:nw], w1[:, c, :],
                                h2T[:, c, g0 * P:(g0 + gn) * P],
                                start=(c == 0), stop=(c == C4 - 1))
                        relu = ptile.tile([P, 512], F32R, tag="relu")
                        nc.scalar.activation(
                            relu[:, 0:nw], pf[:, 0:nw], AF.Relu,
                            bias=fb1_sb[:, l * PJT + p_:l * PJT + p_ + 1])
                        for th in range(gn):
                            nc.tensor.matmul(
                                pf2[:, th, :], relu[:, th * P:(th + 1) * P],
                                w2[:], start=(p_ == 0), stop=(p_ == PJT - 1))
                    for th in range(gn):
                        t = g0 + th
                        nc.vector.tensor_add(out_sb[:, t, :], out_sb[:, t, :],
                                             pf2[:, th, :])

            # ---- boundary head + blend ----
            yout_r = yout_d.rearrange("(t p) n -> p t n", p=P)
            for t in (range(TT) if KSTAGE >= 6 else []):
                lxt = sb.tile([P, HID], F32, tag="lxt")
                nc.sync.dma_start(lxt[:], lx_r[:, t, :])
                if KSTAGE >= 7:
                    z = sb.tile([P, 1], F32, tag="z")
                    zjunk = sb.tile([P, HID], F32, tag="diff")
                    nc.vector.scalar_tensor_tensor(
                        zjunk[:], out_sb[:, t, :], 1.0, bdw_bc[:],
                        op0=ALU.mult, op1=ALU.mult, accum_out=z[:])
                    msk = sb.tile([P, 1], F32, tag="msk")
                    if KSTAGE >= 8:
                        nc.vector.tensor_single_scalar(
                            msk[:], z[:], float(-bd_b_val), op=ALU.is_gt)
                    else:
                        nc.vector.tensor_copy(msk[:], z[:])
                    nc.sync.dma_start(ymask_d[:, t:t + 1], msk[:])
                diff = sb.tile([P, HID], F32, tag="diff")
                nc.vector.tensor_sub(diff[:], out_sb[:, t, :], lxt[:])
                res = sb.tile([P, HID], F32, tag="res")
                nc.vector.scalar_tensor_tensor(
                    res[:], diff[:], mm_sb[:, t:t + 1], lxt[:],
                    op0=ALU.mult, op1=ALU.add)
                nc.sync.dma_start(yout_r[:, t, :], res[:])

    nc.compile()
    return nc


_CACHE: dict = {}


def kernel(layer_x, full_mask, mask_multiplier, qkv_w, qkv_b, o_w, o_b,
           ln1_g, ln1_b, ff_w1, ff_b1, ff_w2, ff_b2, ln2_g, ln2_b,
           bd_w, bd_b):
    layer_x = np.asarray(layer_x, dtype=np.float32)
    mask_multiplier = np.asarray(mask_multiplier, dtype=np.float32)
    qkv_w = np.asarray(qkv_w, dtype=np.float32)
    qkv_b = np.asarray(qkv_b, dtype=np.float32)
    o_w = np.asarray(o_w, dtype=np.float32)
    o_b = np.asarray(o_b, dtype=np.float32)
    ln1_g = np.asarray(ln1_g, dtype=np.float32)
    ln1_b = np.asarray(ln1_b, dtype=np.float32)
    ff_w1 = np.asarray(ff_w1, dtype=np.float32)
    ff_b1 = np.asarray(ff_b1, dtype=np.float32)
    ff_w2 = np.asarray(ff_w2, dtype=np.float32)
    ff_b2 = np.asarray(ff_b2, dtype=np.float32)
    ln2_g = np.asarray(ln2_g, dtype=np.float32)
    ln2_b = np.asarray(ln2_b, dtype=np.float32)
    bd_w = np.asarray(bd_w, dtype=np.float32)
    bd_b = np.asarray(bd_b, dtype=np.float32)

    # host-side folding (exact in fp32/fp64 where it matters)
    qkv_w_eff = np.empty_like(qkv_w)
    qkv_b_eff = np.empty_like(qkv_b)
    fw1_eff = np.empty_like(ff_w1)
    fb1_eff = np.empty_like(ff_b1)
    o_b_eff = np.empty_like(o_b)
    for i in range(NL):
        qkv_w_eff[i] = ln1_g[i][:, None] * qkv_w[i]
        qkv_b_eff[i] = qkv_b[i] + ln1_b[i] @ qkv_w[i]
        fw1_eff[i] = ln2_g[i][:, None] * ff_w1[i]
        fb1_eff[i] = ff_b1[i] + ln2_b[i] @ ff_w1[i]
        # V bias folds through the softmax (rows sum to 1) into o_b
        o_b_eff[i] = o_b[i] + qkv_b_eff[i][2 * QKD:] @ o_w[i]
    if np.any(o_b_eff != 0.0) or np.any(ff_b2 != 0.0):
        raise NotImplementedError(
            "nonzero output-projection/FFN2 bias not supported by this kernel")

    qkb_host = np.concatenate(
        [qkv_b_eff[i][:2 * QKD].reshape(8, P).T for i in range(NL)],
        axis=1).astype(np.float32)                      # [128, NL*8]
    fb1_host = np.concatenate(
        [fb1_eff[i].reshape(PJT, P).T for i in range(NL)],
        axis=1).astype(np.float32)                      # [128, NL*16]

    key = float(bd_b[0])
    if key not in _CACHE:
        _CACHE[key] = _build(key)
    nc = _CACHE[key]

    def pmajor(w):
        # [HID, N] -> [128, C4*N] partition-major for contiguous DMA
        n = w.shape[1]
        return np.ascontiguousarray(
            w.reshape(C4, P, n).transpose(1, 0, 2).reshape(P, C4 * n))

    qkvw_h = np.stack([pmajor(qkv_w_eff[i]) for i in range(NL)])
    ow_h = np.stack([pmajor(o_w[i]) for i in range(NL)])
    fw1_h = np.stack([
        np.stack([pmajor(fw1_eff[i][:, p * P:(p + 1) * P]) for p in range(PJT)])
        for i in range(NL)])
    shared = {
        "bdw": np.ascontiguousarray(bd_w.reshape(1, HID)),
        "qkvw": _round_f32r(qkvw_h),
        "qkb": np.ascontiguousarray(qkb_host),
        "ow": _round_f32r(ow_h),
        "fw1": _round_f32r(fw1_h),
        "fb1": np.ascontiguousarray(fb1_host),
        "fw2": _round_f32r(ff_w2),
    }
    in_maps = []
    for b in range(B):
        m = dict(shared)
        m["lx"] = np.ascontiguousarray(layer_x[b])
        m["mmul"] = np.ascontiguousarray(
            mask_multiplier[b, :, 0].reshape(TT, P).T)
        in_maps.append(m)

    res = run_bass_kernel_spmd(nc, in_maps, core_ids=list(range(N_CORES)))
    global LAST_RESULT
    LAST_RESULT = res
    new_out = np.stack([res.results[b]["yout"] for b in range(B)])
    new_mask = np.stack(
        [res.results[b]["ymask"].T.reshape(L, 1) for b in range(B)])
    return new_out, new_mask


# revision 21
# speedup vs baseline: 1.0560x; 1.0057x over previous
"""Trainium2 Bass kernel for nn_BoundaryTransformer_Block (2-layer causal
transformer + boundary-decision head), data-parallel over batch on 4 cores.

Strategy
--------
- Core c computes batch element c fully (B=4). No collectives (host-side
  measurements show collectives on this rig are host-bounced and ~ms-slow).
- All matmuls run as float32r (full PE rate at N>=256). Weights are
  pre-rounded to the f32r grid on host; on-device producers write f32r
  directly so the bir verifier accepts them as matmul operands.
- Residual stream kept in natural layout [128 tok, 512 hid] per 128-token
  tile; LayerNorm reduces along the free dim. The LN output is transposed
  via PE (identity matmul) into [hid, tok] for the QKV projections.
- Attention computed per (head, 512-query-half) with scores produced
  TRANSPOSED: S^T[kp, qp] = k^T(lhsT) x q^T(rhs). Softmax skips the max
  subtraction (scores are O(1) here); the row sum is obtained for free by
  augmenting V with a ones column (65th lhsT row of the av matmul).
- Causal structure: only lower-triangular 128x128 blocks are computed;
  the diagonal block is masked by an upper-triangular 0/1 multiply.
- LN gains/biases are folded into the following projection weights on the
  host; the V bias is folded into the output-projection bias (softmax rows
  sum to 1). Boundary mask = (out @ bd_w + bd_b > 0) exactly.
"""
import sys
import os
import numpy as np

for _p in ("/opt/trn_rl_repo",):
    if _p not in sys.path:
        sys.path.insert(0, _p)

# bass_utils' axon trace path imports antenv.axon_hooks, which this image
# lacks; register a no-op fallback so a stray BASS_TRACE=1 cannot crash us.
try:
    import antenv.axon_hooks  # noqa: F401
except ImportError:
    import types as _types
    if "antenv.axon_hooks" not in sys.modules:
        _m = _types.ModuleType("antenv.axon_hooks")
        _m.get_axon_ntff_profile_hook = lambda: None
        _m.set_axon_ntff_profile_hook = lambda h: None
        sys.modules["antenv.axon_hooks"] = _m

import concourse.bass as bass
import concourse.mybir as mybir
import concourse.tile as tile
from concourse import bacc
from concourse.bass_utils import run_bass_kernel_spmd
from concourse.masks import make_identity, make_upper_triangular

B, L, HID, NH, HD, PROJ, NL = 4, 1024, 512, 8, 64, 2048, 2
LN_EPS = 1e-5
P = 128
TT = L // P        # 8 token tiles
C4 = HID // P      # 4 hidden chunks
PJT = PROJ // P    # 16 proj tiles
QKD = NH * HD      # 512
F32 = mybir.dt.float32
F32R = mybir.dt.float32r
N_CORES = 4
AF = mybir.ActivationFunctionType
ALU = mybir.AluOpType


def _round_f32r(a: np.ndarray) -> np.ndarray:
    """Round fp32 values to the fp32r grid (RNE at 12 mantissa LSBs)."""
    u = np.ascontiguousarray(a, dtype=np.float32).view(np.uint32)
    r = (u.astype(np.uint64) + 0x7FF + ((u >> 12) & 1)) & 0xFFFFF000
    return np.ascontiguousarray(r.astype(np.uint32)).view(np.float32)


def _layer_norm_to_hT(nc, sb, ps, xT_pool, out_sb, ident_r, eps_ap=None):
    """LN(out_sb) along free dim, write transposed result [128, C4, L] f32r.

    out_sb: [128, TT, HID] f32 residual. Returns hT tile [128, C4, TT*P] f32r.
    """
    hT = xT_pool.tile([P, C4, L], F32R, tag="xT")
    for t in range(TT):
        x = out_sb[:, t, :]
        stats = sb.tile([P, 6], F32, tag="lnstat")
        nc.vector.bn_stats(stats[:], x)
        mv = sb.tile([P, 2], F32, tag="lnmv")
        nc.vector.bn_aggr(mv[:], stats[:])
        sd = sb.tile([P, 1], F32, tag="lnsd")
        nc.scalar.activation(sd[:], mv[:, 1:2], AF.Sqrt, bias=eps_ap, scale=1.0)
        rstd = sb.tile([P, 1], F32, tag="lnrstd")
        nc.vector.reciprocal(rstd[:], sd[:])
        h = sb.tile([P, HID], F32R, tag="h")
        nc.vector.tensor_scalar(h[:], x, mv[:, 0:1], rstd[:],
                                op0=ALU.subtract, op1=ALU.mult)
        for c in range(C4):
            ptr = ps.tile([P, P], F32R, tag="ps_a")
            nc.tensor.transpose(ptr[:], h[:, c * P:(c + 1) * P], ident_r[:])
            nc.vector.tensor_copy(hT[:, c, t * P:(t + 1) * P], ptr[:])
    return hT


KSTAGE = int(os.environ.get("KSTAGE", "9"))


def _build(bd_b_val: float):
    nc = bacc.Bacc("TRN2", target_bir_lowering=False, debug=False,
                   num_devices=N_CORES)

    lx_d = nc.dram_tensor("lx", [L, HID], F32, kind="ExternalInput")
    mm_d = nc.dram_tensor("mmul", [P, TT], F32, kind="ExternalInput")
    bdw_d = nc.dram_tensor("bdw", [1, HID], F32, kind="ExternalInput")
    qkvw_d = nc.dram_tensor("qkvw", [NL, P, C4 * 3 * QKD], F32R, kind="ExternalInput")
    qkb_d = nc.dram_tensor("qkb", [P, NL * 8], F32, kind="ExternalInput")
    ow_d = nc.dram_tensor("ow", [NL, P, C4 * HID], F32R, kind="ExternalInput")
    fw1_d = nc.dram_tensor("fw1", [NL, PJT, P, C4 * P], F32R, kind="ExternalInput")
    fb1_d = nc.dram_tensor("fb1", [P, NL * PJT], F32, kind="ExternalInput")
    fw2_d = nc.dram_tensor("fw2", [NL, PROJ, HID], F32R, kind="ExternalInput")
    yout_d = nc.dram_tensor("yout", [L, HID], F32, kind="ExternalOutput")
    ymask_d = nc.dram_tensor("ymask", [P, TT], F32, kind="ExternalOutput")

    with tile.TileContext(nc) as tc:
        with tc.tile_pool(name="const", bufs=1) as cpool, \
             tc.tile_pool(name="sb", bufs=2) as sb, \
             tc.tile_pool(name="big", bufs=1) as big, \
             tc.tile_pool(name="xT", bufs=2) as xT_pool, \
             tc.tile_pool(name="wpool", bufs=1) as wpool, \
             tc.tile_pool(name="wstream", bufs=3) as wstream, \
             tc.tile_pool(name="pt", bufs=3) as ptile, \
             tc.tile_pool(name="ps", bufs=2, space="PSUM") as ps, \
             tc.tile_pool(name="psb", bufs=2, space="PSUM") as psb, \
             tc.tile_pool(name="ps4", bufs=1, space="PSUM") as ps4:

            # ---- constants ----
            ident = cpool.tile([P, P], F32)
            make_identity(nc, ident[:])
            mtri = cpool.tile([P, P], F32)
            make_upper_triangular(nc, mtri[:], val=1.0, diag=True)
            qkb_sb = cpool.tile([P, NL * 8], F32)
            nc.sync.dma_start(qkb_sb[:], qkb_d[:])
            fb1_sb = cpool.tile([P, NL * PJT], F32)
            nc.sync.dma_start(fb1_sb[:], fb1_d[:])
            bdw_sb = cpool.tile([1, HID], F32)
            nc.sync.dma_start(bdw_sb[:], bdw_d[:])
            bdw_bc = cpool.tile([P, HID], F32)
            nc.gpsimd.partition_broadcast(bdw_bc[:], bdw_sb[:])
            mm_sb = cpool.tile([P, TT], F32)
            nc.sync.dma_start(mm_sb[:], mm_d[:])
            ident_r = cpool.tile([P, P], F32R)
            nc.scalar.copy(ident_r[:], ident[:])
            eps_sb = cpool.tile([P, 1], F32)
            nc.vector.memset(eps_sb[:], LN_EPS)
            ones64_r = cpool.tile([P, HD], F32R)
            nc.scalar.activation(ones64_r[:], ident[:, 0:HD], AF.Identity,
                                 bias=1.0, scale=0.0)

            # ---- residual init ----
            out_sb = big.tile([P, TT, HID], F32, tag="resid")
            lx_r = lx_d.rearrange("(t p) n -> p t n", p=P)
            nc.sync.dma_start(out_sb[:], lx_r)

            for l in range(NL):
                # ---- layer weights ----
                qkvw_sb = wpool.tile([P, C4, 3 * QKD], F32R, tag="qkvw")
                nc.sync.dma_start(qkvw_sb[:].rearrange("p c n -> p (c n)"),
                                  qkvw_d[l])
                ow_sb = wpool.tile([P, C4, HID], F32R, tag="ow")
                nc.sync.dma_start(ow_sb[:].rearrange("p c n -> p (c n)"),
                                  ow_d[l])

                # ---- LN1 + transpose ----
                hT = _layer_norm_to_hT(nc, sb, ps, xT_pool, out_sb, ident_r, eps_sb[:])

                # ---- QKV projections ----
                # q^T, k^T: [128, 8, 1024] f32r; block m in 0..7 (q: 0-3, k: 4-7)
                qkT = big.tile([P, 8, L], F32R, tag="qkT")
                for m in (range(8) if KSTAGE >= 2 else []):
                    for half in range(2):
                        pq = ps.tile([P, 512], F32, tag="ps_a")
                        for c in range(C4):
                            nc.tensor.matmul(
                                pq[:], qkvw_sb[:, c, m * P:(m + 1) * P],
                                hT[:, c, half * 512:(half + 1) * 512],
                                start=(c == 0), stop=(c == C4 - 1))
                        nc.vector.tensor_copy(
                            qkT[:, m, half * 512:(half + 1) * 512], pq[:])
                # v (natural) + ones column: [128, NH, HD+1] per token tile
                v_aug = []
                for t in (range(TT) if KSTAGE >= 2 else []):
                    va = big.tile([P, NH, HD + 1], F32R, tag=f"vaug{t}")
                    pv = ps.tile([P, 512], F32, tag="ps_a")
                    for c in range(C4):
                        nc.tensor.matmul(
                            pv[:], hT[:, c, t * P:(t + 1) * P],
                            qkvw_sb[:, c, 2 * QKD:3 * QKD],
                            start=(c == 0), stop=(c == C4 - 1))
                    nc.vector.tensor_copy(
                        va[:, :, 0:HD], pv[:].rearrange("p (h d) -> p h d", h=NH))
                    nc.scalar.activation(va[:, :, HD:HD + 1], mtri[:, 0:NH],
                                         AF.Identity, bias=1.0, scale=0.0)
                    v_aug.append(va)

                # ---- attention ----
                avT = xT_pool.tile([P, C4, L], F32R, tag="xT")

                def emit_norm(h, qh, pav):
                    # normalize av rows by the ones-row sum; deferred by one
                    # (h, qh) iteration so its latency hides under the next
                    # head's matmuls.
                    r0 = (h % 2) * HD
                    mb = h // 2
                    srow = sb.tile([P, 512], F32, tag="srow")
                    nc.vector.tensor_copy(srow[HD:HD + 1, :], pav[HD:HD + 1, :])
                    sq = sb.tile([P, 4], F32, tag="sq")
                    nc.sync.dma_start(sq[:], srow[HD:HD + 1, :])
                    sqr = sb.tile([P, 4], F32, tag="sqr")
                    nc.vector.reciprocal(sqr[:], sq[:])
                    srow0 = sb.tile([1, 512], F32, tag="srow0")
                    nc.sync.dma_start(srow0[:], sqr[:])
                    bc_sb = sb.tile([HD, 512], F32, tag="bcsb")
                    nc.gpsimd.partition_broadcast(bc_sb[:], srow0[:])
                    if r0 == 0:
                        nc.vector.tensor_mul(
                            avT[0:HD, mb, qh * 512:(qh + 1) * 512],
                            pav[0:HD, :], bc_sb[:])
                    else:
                        # DVE is lane-locked; compute at base 0 and let a
                        # SBUF->SBUF DMA shift the result up 64 partitions.
                        avtmp = sb.tile([HD, 512], F32R, tag="avtmp")
                        nc.vector.tensor_mul(avtmp[:], pav[0:HD, :], bc_sb[:])
                        nc.sync.dma_start(
                            avT[HD:P, mb, qh * 512:(qh + 1) * 512], avtmp[:])

                pending = None
                for h in (range(NH) if KSTAGE >= 3 else []):
                    r0 = (h % 2) * HD        # partition offset of this head
                    mb = h // 2              # 128-block of q/k dims
                    for qh in range(2):
                        pav = psb.tile([P, 512], F32, tag="pav")
                        jmax = 4 * (qh + 1)

                        def emit_S(j):
                            qlo = max(j - 4 * qh, 0) * P      # local col start
                            n = 512 - qlo
                            pS = ps.tile([P, 512], F32, tag="ps_a")
                            nc.tensor.matmul(
                                pS[:, 0:n],
                                qkT[r0:r0 + HD, 4 + mb, j * P:(j + 1) * P],
                                qkT[r0:r0 + HD, mb, qh * 512 + qlo:(qh + 1) * 512],
                                start=True, stop=True)
                            return pS, qlo, n

                        def emit_av(j, pS, qlo, n):
                            pt_ = ptile.tile([P, 512], F32R, tag="ptile")
                            nc.scalar.activation(pt_[:, 0:n], pS[:, 0:n],
                                                 AF.Exp, scale=float(1.0 / np.sqrt(HD)))
                            if qh == 0 or j >= 4:
                                nc.vector.tensor_mul(pt_[:, 0:P], pt_[:, 0:P],
                                                     mtri[:])
                            nc.tensor.matmul(
                                pav[0:HD + 1, qlo:512],
                                v_aug[j][:, h, :], pt_[:, 0:n],
                                start=(j == 0), stop=(j == jmax - 1))

                        prev = None
                        for j in range(jmax):
                            cur = (j,) + emit_S(j)
                            if prev is not None:
                                emit_av(*prev)
                            prev = cur
                        emit_av(*prev)
                        if pending is not None:
                            emit_norm(*pending)
                        pending = (h, qh, pav)
                if pending is not None:
                    emit_norm(*pending)
                    pending = None

                # ---- output projection + residual ----
                for t in (range(TT) if KSTAGE >= 4 else []):
                    po = ps.tile([P, 512], F32, tag="ps_a")
                    for c in range(C4):
                        nc.tensor.matmul(po[:], avT[:, c, t * P:(t + 1) * P],
                                         ow_sb[:, c, :],
                                         start=(c == 0), stop=(c == C4 - 1))
                    nc.vector.tensor_add(out_sb[:, t, :], out_sb[:, t, :], po[:])

                # ---- LN2 + transpose ----
                h2T = _layer_norm_to_hT(nc, sb, ps, xT_pool, out_sb, ident_r, eps_sb[:])

                # ---- FFN ----
                groups = [(0, 4), (4, 4)]
                for g0, gn in (groups if KSTAGE >= 5 else []):
                    pf2 = ps4.tile([P, 4, 512], F32, tag="ps_out2")
                    for p_ in range(PJT):
                        w1 = wstream.tile([P, C4, P], F32R, tag="w1")
                        nc.sync.dma_start(
                            w1[:].rearrange("p c n -> p (c n)"), fw1_d[l, p_])
                        w2 = wstream.tile([P, HID], F32R, tag="w2")
                        nc.sync.dma_start(w2[:], fw2_d[l, p_ * P:(p_ + 1) * P, :])
                        pf = ps.tile([P, 512], F32, tag="ps_a")
                        nw = gn * P
                        for c in range(C4):
                            nc.tensor.matmul(
                                pf[:, 0:nw], w1[:, c, :],
                                h2T[:, c, g0 * P:(g0 + gn) * P],
                                start=(c == 0), stop=(c == C4 - 1))
                        relu = ptile.tile([P, 512], F32R, tag="relu")
                        nc.scalar.activation(
                            relu[:, 0:nw], pf[:, 0:nw], AF.Relu,
                            bias=fb1_sb[:, l * PJT + p_:l * PJT + p_ + 1])
                        for th in range(gn):
                            nc.tensor.matmul(
                                pf2[:, th, :], relu[:, th * P:(th + 1) * P],
                                w2[:], start=(p_ == 0), stop=(p_ == PJT - 1))
                    for th in range(gn):
                        t = g0 + th
                        nc.vector.tensor_add(out_sb[:, t, :], out_sb[:, t, :],
                                             pf2[:, th, :])

            # ---- boundary head + blend ----
            yout_r = yout_d.rearrange("(t p) n -> p t n", p=P)
            for t in (range(TT) if KSTAGE >= 6 else []):
                lxt = sb.tile([P, HID], F32, tag="lxt")
                nc.sync.dma_start(lxt[:], lx_r[:, t, :])
                if KSTAGE >= 7:
                    z = sb.tile([P, 1], F32, tag="z")
                    zjunk = sb.tile([P, HID], F32, tag="diff")
                    nc.vector.scalar_tensor_tensor(
                        zjunk[:], out_sb[:, t, :], 1.0, bdw_bc[:],
                        op0=ALU.mult, op1=ALU.mult, accum_out=z[:])
                    msk = sb.tile([P, 1], F32, tag="msk")
                    if KSTAGE >= 8:
                        nc.vector.tensor_single_scalar(
                            msk[:], z[:], float(-bd_b_val), op=ALU.is_gt)
                    else:
                        nc.vector.tensor_copy(msk[:], z[:])
                    nc.sync.dma_start(ymask_d[:, t:t + 1], msk[:])
                diff = sb.tile([P, HID], F32, tag="diff")
                nc.vector.tensor_sub(diff[:], out_sb[:, t, :], lxt[:])
                res = sb.tile([P, HID], F32, tag="res")
                nc.vector.scalar_tensor_tensor(
                    res[:], diff[:], mm_sb[:, t:t + 1], lxt[:],
                    op0=ALU.mult, op1=ALU.add)
                nc.sync.dma_start(yout_r[:, t, :], res[:])

    nc.compile()
    return nc


_CACHE: dict = {}


def kernel(layer_x, full_mask, mask_multiplier, qkv_w, qkv_b, o_w, o_b,
           ln1_g, ln1_b, ff_w1, ff_b1, ff_w2, ff_b2, ln2_g, ln2_b,
           bd_w, bd_b):
    layer_x = np.asarray(layer_x, dtype=np.float32)
    mask_multiplier = np.asarray(mask_multiplier, dtype=np.float32)
    qkv_w = np.asarray(qkv_w, dtype=np.float32)
    qkv_b = np.asarray(qkv_b, dtype=np.float32)
    o_w = np.asarray(o_w, dtype=np.float32)
    o_b = np.asarray(o_b, dtype=np.float32)
    ln1_g = np.asarray(ln1_g, dtype=np.float32)
    ln1_b = np.asarray(ln1_b, dtype=np.float32)
    ff_w1 = np.asarray(ff_w1, dtype=np.float32)
    ff_b1 = np.asarray(ff_b1, dtype=np.float32)
    ff_w2 = np.asarray(ff_w2, dtype=np.float32)
    ff_b2 = np.asarray(ff_b2, dtype=np.float32)
    ln2_g = np.asarray(ln2_g, dtype=np.float32)
    ln2_b = np.asarray(ln2_b, dtype=np.float32)
    bd_w = np.asarray(bd_w, dtype=np.float32)
    bd_b = np.asarray(bd_b, dtype=np.float32)

    # host-side folding (exact in fp32/fp64 where it matters)
    qkv_w_eff = np.empty_like(qkv_w)
    qkv_b_eff = np.empty_like(qkv_b)
    fw1_eff = np.empty_like(ff_w1)
    fb1_eff = np.empty_like(ff_b1)
    o_b_eff = np.empty_like(o_b)
    for i in range(NL):
        qkv_w_eff[i] = ln1_g[i][:, None] * qkv_w[i]
        qkv_b_eff[i] = qkv_b[i] + ln1_b[i] @ qkv_w[i]
        fw1_eff[i] = ln2_g[i][:, None] * ff_w1[i]
        fb1_eff[i] = ff_b1[i] + ln2_b[i] @ ff_w1[i]
        # V bias folds through the softmax (rows sum to 1) into o_b
        o_b_eff[i] = o_b[i] + qkv_b_eff[i][2 * QKD:] @ o_w[i]
    if np.any(o_b_eff != 0.0) or np.any(ff_b2 != 0.0):
        raise NotImplementedError(
            "nonzero output-projection/FFN2 bias not supported by this kernel")

    qkb_host = np.concatenate(
        [qkv_b_eff[i][:2 * QKD].reshape(8, P).T for i in range(NL)],
        axis=1).astype(np.float32)                      # [128, NL*8]
    fb1_host = np.concatenate(
        [fb1_eff[i].reshape(PJT, P).T for i in range(NL)],
        axis=1).astype(np.float32)                      # [128, NL*16]

    key = float(bd_b[0])
    if key not in _CACHE:
        _CACHE[key] = _build(key)
    nc = _CACHE[key]

    def pmajor(w):
        # [HID, N] -> [128, C4*N] partition-major for contiguous DMA
        n = w.shape[1]
        return np.ascontiguousarray(
            w.reshape(C4, P, n).transpose(1, 0, 2).reshape(P, C4 * n))

    qkvw_h = np.stack([pmajor(qkv_w_eff[i]) for i in range(NL)])
    ow_h = np.stack([pmajor(o_w[i]) for i in range(NL)])
    fw1_h = np.stack([
        np.stack([pmajor(fw1_eff[i][:, p * P:(p + 1) * P]) for p in range(PJT)])
        for i in range(NL)])
    shared = {
        "bdw": np.ascontiguousarray(bd_w.reshape(1, HID)),
        "qkvw": _round_f32r(qkvw_h),
        "qkb": np.ascontiguousarray(qkb_host),
        "ow": _round_f32r(ow_h),
        "fw1": _round_f32r(fw1_h),
        "fb1": np.ascontiguousarray(fb1_host),
        "fw2": _round_f32r(ff_w2),
    }
    in_maps = []
    for b in range(B):
        m = dict(shared)
        m["lx"] = np.ascontiguousarray(layer_x[b])
        m["mmul"] = np.ascontiguousarray(
            mask_multiplier[b, :, 0].reshape(TT, P).T)
        in_maps.append(m)

    res = run_bass_kernel_spmd(nc, in_maps, core_ids=list(range(N_CORES)))
    global LAST_RESULT
    LAST_RESULT = res
    new_out = np.stack([res.results[b]["yout"] for b in range(B)])
    new_mask = np.stack(
        [res.results[b]["ymask"].T.reshape(L, 1) for b in range(B)])
    return new_out, new_mask
